# revision 3
# baseline (speedup 1.0000x reference)
"""AliNet graph-attention layer on 8 Trainium2 NeuronCores.

Pipeline (3 SPMD launches, host does sharding glue only):
  L1: per-core BN partial sums over its node slice  -> host combines stats
  L2: per-core node phase: xn = BN(x), mapped = xn@K, s1/s2 = tanh(rowdot)
  host: packs the gather table (mapped rows with s2 spliced into mantissa
        LSBs of cols 125..127) and replicates it to every core
  L3: per-core edge phase over its src-owned edges (sorted by src window,
      bucketed by dst chunk for int16 dma_gather):
        gather rows of table by dst, build one-hot of src_rel on DVE,
        w = exp(leaky_relu(s1[src]+s2[dst])), scatter-add via PE matmul
        acc[:,0:128] += S01^T @ (G*w),  acc[:,128] += S01^T @ w
      out = relu(acc[:,0:128] / max(acc[:,128], 1e-16))
"""

import math
import numpy as np
import ml_dtypes

import concourse.bass as bass
import concourse.bacc as bacc
import concourse.tile as tile
import concourse.mybir as mybir
import concourse.bass_utils as bass_utils

F32 = mybir.dt.float32
BF16 = mybir.dt.bfloat16
I16 = mybir.dt.int16
I32 = mybir.dt.int32
AF = mybir.ActivationFunctionType
OP = mybir.AluOpType

BN_EPS = 1e-5
P = 128

RUN_MODE = "hw"  # "hw" or "sim"
L3_F32 = False


class Cfg:
    def __init__(self, N=100000, D=128, NC=8, CHUNK=25600, GW=4):
        self.N, self.D, self.NC = N, D, NC
        assert N % NC == 0
        self.NS = N // NC                    # nodes per core
        self.NW = math.ceil(self.NS / P)     # src windows per core
        self.CHUNK = CHUNK                   # dst chunk (int16 idx range)
        self.NCH = math.ceil(N / CHUNK)      # dst chunks
        self.GW = GW                         # windows per gather group
        self.NG = math.ceil(self.NW / GW)


CFG = Cfg()


def _mk_nc(num_devices):
    return bacc.Bacc(
        "TRN2",
        target_bir_lowering=False,
        debug=False,
        enable_asserts=True,
        num_devices=num_devices,
    )


# ---------------------------------------------------------------- L1: stats
def build_l1(cfg):
    nc = _mk_nc(cfg.NC)
    x = nc.dram_tensor("x_slice", [cfg.NS, cfg.D], F32, kind="ExternalInput")
    stats = nc.dram_tensor("stats", [cfg.D, 2], F32, kind="ExternalOutput")
    ntiles = math.ceil(cfg.NS / P)
    with tile.TileContext(nc) as tc:
        with (
            tc.tile_pool(name="sb", bufs=3) as sb,
            tc.tile_pool(name="cst", bufs=1) as cst,
            tc.tile_pool(name="ps", bufs=1, space="PSUM") as ps,
        ):
            ones = cst.tile([P, 1], F32)
            nc.gpsimd.memset(ones[:], 1.0)
            acc0 = ps.tile([cfg.D, 1], F32, tag="a0")
            acc1 = ps.tile([cfg.D, 1], F32, tag="a1")
            for t in range(ntiles):
                r0 = t * P
                rows = min(P, cfg.NS - r0)
                xt = sb.tile([P, cfg.D], F32, tag="xt")
                nc.sync.dma_start(xt[:rows, :], x[r0 : r0 + rows, :])
                xsq = sb.tile([P, cfg.D], F32, tag="xsq")
                nc.vector.tensor_tensor(
                    out=xsq[:rows, :], in0=xt[:rows, :], in1=xt[:rows, :], op=OP.mult
                )
                nc.tensor.matmul(
                    acc0[:, 0:1], xt[:rows, :], ones[:rows, :],
                    start=(t == 0), stop=(t == ntiles - 1),
                )
                nc.tensor.matmul(
                    acc1[:, 0:1], xsq[:rows, :], ones[:rows, :],
                    start=(t == 0), stop=(t == ntiles - 1),
                )
            out_sb = sb.tile([cfg.D, 2], F32, tag="o")
            nc.vector.tensor_copy(out_sb[:, 0:1], acc0[:])
            nc.vector.tensor_copy(out_sb[:, 1:2], acc1[:])
            nc.sync.dma_start(stats[:], out_sb[:])
    nc.compile()
    return nc


# ------------------------------------------------------------ L2: node phase
def build_l2(cfg):
    nc = _mk_nc(cfg.NC)
    D, NS, NW = cfg.D, cfg.NS, cfg.NW
    xT = nc.dram_tensor("xT_slice", [D, NS], F32, kind="ExternalInput")
    scale = nc.dram_tensor("scale", [D, 1], F32, kind="ExternalInput")
    shift = nc.dram_tensor("shift", [D, 1], F32, kind="ExternalInput")
    k0 = nc.dram_tensor("k0", [D, D], F32, kind="ExternalInput")
    k1 = nc.dram_tensor("k1", [D, D], F32, kind="ExternalInput")
    k2 = nc.dram_tensor("k2", [D, D], F32, kind="ExternalInput")
    mappedT = nc.dram_tensor("mappedT", [D, NS], F32, kind="ExternalOutput")
    s1o = nc.dram_tensor("s1o", [P, NW], F32, kind="ExternalOutput")
    s2o = nc.dram_tensor("s2o", [P, NW], F32, kind="ExternalOutput")

    with tile.TileContext(nc) as tc:
        with (
            tc.tile_pool(name="cst", bufs=1) as cst,
            tc.tile_pool(name="sb", bufs=3) as sb,
            tc.tile_pool(name="ps", bufs=2, space="PSUM") as ps,
            tc.tile_pool(name="ps1", bufs=2, space="PSUM") as ps1,
        ):
            ksb = cst.tile([D, D], F32, tag="k0")
            k1sb = cst.tile([D, D], F32, tag="k1")
            k2sb = cst.tile([D, D], F32, tag="k2")
            ssb = cst.tile([D, 1], F32, tag="sc")
            bsb = cst.tile([D, 1], F32, tag="sh")
            ones = cst.tile([D, 1], F32, tag="on")
            s1sb = cst.tile([P, NW], F32, tag="s1")
            s2sb = cst.tile([P, NW], F32, tag="s2")
            nc.sync.dma_start(ksb[:], k0[:])
            nc.sync.dma_start(k1sb[:], k1[:])
            nc.sync.dma_start(k2sb[:], k2[:])
            nc.sync.dma_start(ssb[:], scale[:])
            nc.sync.dma_start(bsb[:], shift[:])
            nc.gpsimd.memset(ones[:], 1.0)
            nc.gpsimd.memset(s1sb[:], 0.0)
            nc.gpsimd.memset(s2sb[:], 0.0)

            for t in range(NW):
                c0 = t * P
                cols = min(P, NS - c0)
                xt = sb.tile([D, P], F32, tag="xt")
                nc.sync.dma_start(xt[:, :cols], xT[:, c0 : c0 + cols])
                xn = sb.tile([D, P], F32, tag="xn")
                nc.scalar.activation(
                    out=xn[:, :cols], in_=xt[:, :cols], func=AF.Identity,
                    bias=bsb[:, 0:1], scale=ssb[:, 0:1],
                )
                mps = ps.tile([D, P], F32, tag="mm")
                nc.tensor.matmul(mps[:, :cols], ksb[:], xn[:, :cols],
                                 start=True, stop=True)
                msb = sb.tile([D, P], F32, tag="ms")
                nc.scalar.copy(out=msb[:, :cols], in_=mps[:, :cols])
                nc.sync.dma_start(mappedT[:, c0 : c0 + cols], msb[:, :cols])
                for (kw, ssl) in ((k1sb, s1sb), (k2sb, s2sb)):
                    yps = ps.tile([D, P], F32, tag="mm")
                    nc.tensor.matmul(yps[:, :cols], kw[:], xn[:, :cols],
                                     start=True, stop=True)
                    z = sb.tile([D, P], F32, tag="z")
                    nc.vector.tensor_tensor(
                        out=z[:, :cols], in0=yps[:, :cols], in1=xn[:, :cols],
                        op=OP.mult,
                    )
                    sps = ps1.tile([P, 1], F32, tag="s")
                    nc.tensor.matmul(sps[:cols, :], z[:, :cols], ones[:],
                                     start=True, stop=True)
                    nc.scalar.activation(
                        out=ssl[:cols, t : t + 1], in_=sps[:cols, :], func=AF.Tanh
                    )
            nc.sync.dma_start(s1o[:], s1sb[:])
            nc.sync.dma_start(s2o[:], s2sb[:])
    nc.compile()
    return nc


# ------------------------------------------------------------ L3: edge phase
def _gbase(groups, nt, gi, NCH):
    """global tile offset of group gi (in stream order)."""
    return int(
        sum(nt[w][c] for gj in range(gi) for w in groups[gj] for c in range(NCH))
    )


def build_l3(cfg, nt, groups):
    """nt[w][c]: padded tile counts (identical across cores).
    groups: list of lists of window ids."""
    MMDT = F32 if L3_F32 else BF16
    nc = _mk_nc(cfg.NC)
    D, NS, NW, NCH = cfg.D, cfg.NS, cfg.NW, cfg.NCH
    TT = int(sum(nt[w][c] for w in range(NW) for c in range(NCH)))
    IC = TT * P // 16  # idx cols (int16, 16-wrap)

    table = nc.dram_tensor("table", [cfg.N, D], F32, kind="ExternalInput")
    idxs_d = nc.dram_tensor("idxs", [P, max(IC, 1)], I16, kind="ExternalInput")
    srel_d = nc.dram_tensor("srel", [P, max(TT, 1)], F32, kind="ExternalInput")
    s1r_d = nc.dram_tensor("s1rows", [NW, P], F32, kind="ExternalInput")
    out_d = nc.dram_tensor("out", [NS, D], F32, kind="ExternalOutput")

    iota_np = np.broadcast_to(
        np.arange(P, dtype=np.float32), (P, P)
    ).astype(np.float32 if L3_F32 else ml_dtypes.bfloat16)
    iota_dram = nc.inline_tensor(np.ascontiguousarray(iota_np), name="iota_c")
    ones1p_dram = nc.inline_tensor(np.ones((1, P), np.float32), name="ones1p_c")

    # stream order: for g: for c: for w in g: tiles of (w, c)
    gather_calls = {}   # gi -> list of (c, idx_col_off, n_idx, gbuf_tile_off)
    tile_off = {}       # (w, c) -> group-local tile offset
    icol = 0
    for gi, g in enumerate(groups):
        goff = 0
        gather_calls[gi] = []
        for c in range(NCH):
            n_idx = int(sum(nt[w][c] for w in g)) * P
            if n_idx == 0:
                continue
            gather_calls[gi].append((c, icol, n_idx, goff))
            for w in g:
                tile_off[(w, c)] = goff
                goff += int(nt[w][c])
            icol += n_idx // 16
    TG = max(
        int(sum(nt[w][c] for w in g for c in range(NCH))) for g in groups
    )
    GWmax = max(len(g) for g in groups)

    with tile.TileContext(nc) as tc:
        with (
            tc.tile_pool(name="cst", bufs=1) as cst,
            tc.tile_pool(name="gb", bufs=2) as gbp,
            tc.tile_pool(name="s01g", bufs=2) as s01gp,
            tc.tile_pool(name="wb", bufs=2) as wbp,
            tc.tile_pool(name="gw", bufs=4) as gwp,
            tc.tile_pool(name="sm", bufs=4) as smp,
            tc.tile_pool(name="acc", bufs=2, space="PSUM") as accp,
            tc.tile_pool(name="accd", bufs=2, space="PSUM") as accdp,
            tc.tile_pool(name="rps", bufs=2, space="PSUM") as rpsp,
        ):
            iota_bf = cst.tile([P, P], MMDT, tag="iota")
            nc.sync.dma_start(iota_bf[:], iota_dram.ap())
            ones1p = cst.tile([1, P], F32, tag="ones1p")
            nc.sync.dma_start(ones1p[:], ones1p_dram.ap())
            idx_sb = cst.tile([P, max(IC, 1)], I16, tag="idx")
            nc.sync.dma_start(idx_sb[:], idxs_d[:])
            srel_sb = cst.tile([P, max(TT, 1)], F32, tag="srel")
            nc.sync.dma_start(srel_sb[:], srel_d[:])

            for gi, g in enumerate(groups):
                gb0 = _gbase(groups, nt, gi, NCH)
                gbuf = gbp.tile([P, TG, D], F32, tag="gbuf")
                for (c, ic0, n_idx, toff) in gather_calls[gi]:
                    nc.gpsimd.dma_gather(
                        out_ap=gbuf[:, toff : toff + n_idx // P, :],
                        in_ap=table[
                            c * cfg.CHUNK : min((c + 1) * cfg.CHUNK, cfg.N), :
                        ],
                        idxs_ap=idx_sb[:, ic0 : ic0 + n_idx // 16],
                        num_idxs=n_idx,
                        num_idxs_reg=n_idx,
                        elem_size=D,
                        single_packet=False,
                    )

                # replicate s1 rows of this group's windows: repl[:, wi, :]
                repl = s01gp.tile([P, GWmax, P], MMDT, tag="repl")
                for wi, w in enumerate(g):
                    s1row = smp.tile([1, P], F32, tag="s1row")
                    nc.sync.dma_start(s1row[:], s1r_d[w : w + 1, :])
                    rps = rpsp.tile([P, P], F32, tag="rps")
                    nc.tensor.matmul(rps[:], ones1p[:], s1row[:],
                                     start=True, stop=True)
                    nc.vector.tensor_copy(out=repl[:, wi, :], in_=rps[:])

                # one-hots + per-edge s1 via accumulating reduce
                s01g = s01gp.tile([P, TG, P], MMDT, tag="s01")
                tbat = wbp.tile([P, TG, 1], F32, tag="tbat")
                for (c, ic0, n_idx, toff) in gather_calls[gi]:
                    for wi, w in enumerate(g):
                        for k in range(int(nt[w][c])):
                            gt = tile_off[(w, c)] + k
                            nc.vector.tensor_scalar(
                                out=s01g[:, gt, :],
                                in0=iota_bf[:],
                                scalar1=srel_sb[:, gb0 + gt : gb0 + gt + 1],
                                scalar2=None,
                                op0=OP.is_equal,
                            )
                            junk = gwp.tile([P, P], MMDT, tag="junk")
                            nc.vector.scalar_tensor_tensor(
                                out=junk[:],
                                in0=s01g[:, gt, :],
                                scalar=1.0,
                                in1=repl[:, wi, :],
                                op0=OP.mult,
                                op1=OP.mult,
                                accum_out=tbat[:, gt, :],
                            )

                # s2 unpack (bits from cols 125..127) for the whole group
                tg = int(sum(nt[w][c] for w in g for c in range(NCH)))
                gi32 = gbuf[:].bitcast(I32)
                s2i = wbp.tile([P, TG, 1], I32, tag="s2i")
                tmp = wbp.tile([P, TG, 1], I32, tag="tmpi")
                nc.vector.tensor_scalar(
                    out=s2i[:, :tg, :], in0=gi32[:, :tg, 125:126], scalar1=0xFF,
                    scalar2=24, op0=OP.bitwise_and, op1=OP.logical_shift_left,
                )
                nc.vector.tensor_scalar(
                    out=tmp[:, :tg, :], in0=gi32[:, :tg, 126:127], scalar1=0xFF,
                    scalar2=16, op0=OP.bitwise_and, op1=OP.logical_shift_left,
                )
                nc.vector.tensor_tensor(out=s2i[:, :tg, :], in0=s2i[:, :tg, :],
                                        in1=tmp[:, :tg, :], op=OP.bitwise_or)
                nc.vector.tensor_scalar(
                    out=tmp[:, :tg, :], in0=gi32[:, :tg, 127:128], scalar1=0xFF,
                    scalar2=8, op0=OP.bitwise_and, op1=OP.logical_shift_left,
                )
                nc.vector.tensor_tensor(out=s2i[:, :tg, :], in0=s2i[:, :tg, :],
                                        in1=tmp[:, :tg, :], op=OP.bitwise_or)
                # t = s1[src] + s2[dst]; w = exp(lrelu(t))
                wq = wbp.tile([P, TG, 1], F32, tag="wq")
                nc.vector.tensor_tensor(
                    out=wq[:, :tg, :], in0=tbat[:, :tg, :],
                    in1=s2i[:, :tg, :].bitcast(F32), op=OP.add
                )
                nc.vector.scalar_tensor_tensor(
                    out=wq[:, :tg, :], in0=wq[:, :tg, :], scalar=0.01,
                    in1=wq[:, :tg, :], op0=OP.mult, op1=OP.max,
                )
                nc.scalar.activation(out=wq[:, :tg, :], in_=wq[:, :tg, :],
                                     func=AF.Exp)
                wb16 = wbp.tile([P, TG, 1], MMDT, tag="wb16")
                nc.vector.tensor_copy(out=wb16[:, :tg, :], in_=wq[:, :tg, :])
                # round-trip w through bf16 so num/den share identical rounding
                wf = wbp.tile([P, TG, 1], F32, tag="wf")
                nc.vector.tensor_copy(out=wf[:, :tg, :], in_=wb16[:, :tg, :])

                # scatter matmuls per window
                for wi, w in enumerate(g):
                    accn = accp.tile([P, D], F32, tag="acc")
                    accd = accdp.tile([P, 1], F32, tag="accd")
                    tiles_w = [
                        (c, k) for c in range(NCH) for k in range(int(nt[w][c]))
                    ]
                    for i, (c, k) in enumerate(tiles_w):
                        gt = tile_off[(w, c)] + k
                        gw = gwp.tile([P, D], MMDT, tag="gwt")
                        nc.vector.tensor_scalar(
                            out=gw[:], in0=gbuf[:, gt, :],
                            scalar1=wf[:, gt, :], scalar2=None, op0=OP.mult,
                        )
                        st = (i == 0)
                        sp = (i == len(tiles_w) - 1)
                        nc.tensor.matmul(accn[:, 0:D], s01g[:, gt, :], gw[:],
                                         start=st, stop=sp)
                        nc.tensor.matmul(accd[:, 0:1], s01g[:, gt, :],
                                         wb16[:, gt, :], start=st, stop=sp)
                    den = smp.tile([P, 1], F32, tag="den")
                    nc.vector.tensor_scalar(
                        out=den[:], in0=accd[:, 0:1], scalar1=1e-16,
                        scalar2=None, op0=OP.max,
                    )
                    rcp = smp.tile([P, 1], F32, tag="rcp")
                    nc.vector.reciprocal(out=rcp[:], in_=den[:])
                    rows = min(P, NS - w * P)
                    osb = smp.tile([P, D], F32, tag="osb")
                    nc.scalar.activation(
                        out=osb[:rows, :], in_=accn[:rows, 0:D], func=AF.Relu,
                        scale=rcp[:rows, 0:1],
                    )
                    nc.sync.dma_start(out_d[w * P : w * P + rows, :],
                                      osb[:rows, :])
    nc.compile()
    return nc


# ------------------------------------------------------------ host planning
def plan_edges(edge_index, cfg):
    src = np.asarray(edge_index[0], dtype=np.int64)
    dst = np.asarray(edge_index[1], dtype=np.int64)
    NC, NS, NW, NCH, CH = cfg.NC, cfg.NS, cfg.NW, cfg.NCH, cfg.CHUNK
    owner = src // NS
    wloc = (src - owner * NS) // P
    chunk = dst // CH
    key = (owner * NW + wloc) * NCH + chunk
    cnt = np.bincount(key, minlength=NC * NW * NCH).reshape(NC, NW, NCH)
    mx = cnt.max(axis=0)
    padded = ((mx + P - 1) // P) * P
    nt = (padded // P).astype(np.int64)  # [NW, NCH] tile counts
    groups = [list(range(i, min(i + cfg.GW, NW))) for i in range(0, NW, cfg.GW)]

    order = np.argsort(key, kind="stable")
    src_s, dst_s, key_s = src[order], dst[order], key[order]
    bounds = np.searchsorted(key_s, np.arange(NC * NW * NCH + 1))

    streams = []
    for core in range(NC):
        idx_blocks = []
        srel_parts = []
        for g in groups:
            for c in range(NCH):
                call_idx = []
                for w in g:
                    n_pad = int(padded[w, c])
                    if n_pad == 0:
                        continue
                    b = (core * NW + w) * NCH + c
                    lo, hi = bounds[b], bounds[b + 1]
                    d = dst_s[lo:hi] - c * CH
                    s = (src_s[lo:hi] - core * NS) - w * P
                    n_real = hi - lo
                    di = np.zeros(n_pad, np.int16)
                    di[:n_real] = d.astype(np.int16)
                    sr = np.full(n_pad, 200.0, np.float32)
                    sr[:n_real] = s.astype(np.float32)
                    call_idx.append(di)
                    srel_parts.append(sr)
                if call_idx:
                    blk = np.concatenate(call_idx)
                    wrap = blk.reshape(-1, 16).T  # [16, n/16]
                    idx_blocks.append(np.tile(wrap, (8, 1)))
        idx_arr = (
            np.concatenate(idx_blocks, axis=1)
            if idx_blocks else np.zeros((P, 1), np.int16)
        )
        srel = (
            np.concatenate(srel_parts) if srel_parts
            else np.zeros(P, np.float32)
        )
        srel_T = np.ascontiguousarray(srel.reshape(-1, P).T)
        streams.append({"idxs": np.ascontiguousarray(idx_arr), "srel": srel_T})
    return nt, groups, streams


def pack_table(mapped, s2):
    """Splice 24 high bits of s2 (f32) into LSBs of cols 125..127."""
    t = np.ascontiguousarray(mapped.astype(np.float32))
    bits = np.ascontiguousarray(s2.astype("<f4")).view(np.uint32)
    ti = t.view(np.uint32)
    ti[:, 125] = (ti[:, 125] & ~np.uint32(0xFF)) | ((bits >> np.uint32(24)) & np.uint32(0xFF))
    ti[:, 126] = (ti[:, 126] & ~np.uint32(0xFF)) | ((bits >> np.uint32(16)) & np.uint32(0xFF))
    ti[:, 127] = (ti[:, 127] & ~np.uint32(0xFF)) | ((bits >> np.uint32(8)) & np.uint32(0xFF))
    return t


# ------------------------------------------------------------ orchestration
def _run(nc, in_maps, cfg, **kw):
    if RUN_MODE == "sim":
        from concourse.bass_interp import MultiCoreSim

        sim = MultiCoreSim(nc, num_cores=cfg.NC, trace=False)
        for ci, core in enumerate(sim.cores.values()):
            for name, arr in in_maps[ci].items():
                core.tensor(name)[:] = arr
        sim.simulate(check_with_hw=False)
        out_names = []
        for alloc in nc.m.functions[0].allocations:
            if not isinstance(alloc, mybir.MemoryLocationSet):
                continue
            if alloc.kind == "ExternalOutput":
                out_names.append(alloc.memorylocations[0].name)
        results = [
            {n: np.array(core.tensor(n)) for n in out_names}
            for core in sim.cores.values()
        ]

        class R:
            pass

        r = R()
        r.results = results
        r.exec_time_ns = None
        return r
    return bass_utils.run_bass_kernel_spmd(
        nc, in_maps, core_ids=list(range(cfg.NC)), **kw
    )


def kernel(x, edge_index, kernel, kernel1, kernel2, gamma, beta, _cfg=None,
           _trace=False):
    cfg = _cfg or CFG
    x = np.asarray(x, np.float32)
    k0 = np.asarray(kernel, np.float32)
    k1 = np.asarray(kernel1, np.float32)
    k2 = np.asarray(kernel2, np.float32)
    gamma = np.asarray(gamma, np.float32)
    beta = np.asarray(beta, np.float32)
    NC, NS, D = cfg.NC, cfg.NS, cfg.D

    import time as _t
    _ts = _t.time()
    def _lap(msg):
        nonlocal_ns = _t.time()
        print(f"[kernel] {msg}: +{nonlocal_ns - _lap.t0:.1f}s", flush=True)
        _lap.t0 = nonlocal_ns
    _lap.t0 = _ts
    # ---- L1
    nc1 = build_l1(cfg)
    _lap("build_l1")
    in1 = [{"x_slice": np.ascontiguousarray(x[c * NS : (c + 1) * NS])}
           for c in range(NC)]
    r1 = _run(nc1, in1, cfg, trace=_trace)
    _lap("run_l1")
    parts = np.stack([r1.results[c]["stats"] for c in range(NC)])
    tot = parts.sum(axis=0).astype(np.float64)
    mean = tot[:, 0] / cfg.N
    var = tot[:, 1] / cfg.N - mean * mean
    scale = (gamma.astype(np.float64) / np.sqrt(var + BN_EPS)).astype(np.float32)
    shift = (beta.astype(np.float64) - mean * (gamma.astype(np.float64) / np.sqrt(var + BN_EPS))).astype(np.float32)

    # ---- L2
    nc2 = build_l2(cfg)
    _lap("build_l2")
    in2 = []
    for c in range(NC):
        in2.append({
            "xT_slice": np.ascontiguousarray(x[c * NS : (c + 1) * NS].T),
            "scale": np.ascontiguousarray(scale.reshape(D, 1)),
            "shift": np.ascontiguousarray(shift.reshape(D, 1)),
            "k0": k0, "k1": k1, "k2": k2,
        })
    r2 = _run(nc2, in2, cfg, trace=_trace)
    _lap("run_l2")
    mapped = np.concatenate(
        [np.asarray(r2.results[c]["mappedT"]).T for c in range(NC)], axis=0
    )
    s1 = np.concatenate(
        [np.asarray(r2.results[c]["s1o"]).T.reshape(-1)[:NS] for c in range(NC)]
    )
    s2 = np.concatenate(
        [np.asarray(r2.results[c]["s2o"]).T.reshape(-1)[:NS] for c in range(NC)]
    )

    # ---- host glue
    table = pack_table(mapped, s2)
    nt, groups, streams = plan_edges(edge_index, cfg)
    _lap("host_glue")

    # ---- L3
    nc3 = build_l3(cfg, nt, groups)
    _lap("build_l3")
    in3 = []
    for c in range(NC):
        s1pad = np.zeros(cfg.NW * P, np.float32)
        s1pad[:NS] = s1[c * NS : (c + 1) * NS]
        in3.append({
            "table": table,
            "idxs": streams[c]["idxs"],
            "srel": streams[c]["srel"],
            "s1rows": np.ascontiguousarray(s1pad.reshape(cfg.NW, P)),
        })
    r3 = _run(nc3, in3, cfg, trace=_trace)
    _lap("run_l3")
    out = np.concatenate(
        [np.asarray(r3.results[c]["out"]) for c in range(NC)], axis=0
    )
    globals()["_LAST_RESULTS"] = (r1, r2, r3)
    return out



# revision 5
# speedup vs baseline: 1.0469x; 1.0469x over previous
"""AliNet graph-attention layer on 8 Trainium2 NeuronCores (v2).

Pipeline (3 SPMD launches; host does sharding glue + spill edges):
  L1: per-core BN partial sums over its node slice  -> host combines stats
  L2: per-core node phase: xn = BN(x), mapped = xn@K, s1/s2 = tanh(rowdot)
  host: packs table[N, 256] bf16 rows: cols 0:128 mapped, 128 s2, 129 ones;
        buckets edges into fixed 512-slot (window, chunk) cells (overflow
        edges spill to host numpy), builds gather idx + srel streams
  L3: per-core edge phase over its src-owned edges:
        dma_gather rows by dst (row-rate-bound: bytes are free),
        one-hot by srel on DVE, per-edge s1 via replicate+reduce,
        w = exp(leaky_relu(s1+s2)) (Scalar exp), scale one-hot by w,
        ONE matmul per tile accumulates num (cols 0:128) AND den (col 129)
        in PSUM; outputs raw [num|den] per node
  host: adds spill contributions, out = relu(num / max(den, 1e-16))
"""

import math
import numpy as np
import ml_dtypes

import concourse.bass as bass
import concourse.bacc as bacc
import concourse.tile as tile
import concourse.mybir as mybir
import concourse.bass_utils as bass_utils

F32 = mybir.dt.float32
BF16 = mybir.dt.bfloat16
I16 = mybir.dt.int16
I32 = mybir.dt.int32
AF = mybir.ActivationFunctionType
OP = mybir.AluOpType

BN_EPS = 1e-5
P = 128

RUN_MODE = "hw"  # "hw" or "sim"


class Cfg:
    def __init__(self, N=100000, D=128, NC=8, CHUNK=25000, QUOTA=512, GW=4):
        self.N, self.D, self.NC = N, D, NC
        assert N % NC == 0
        self.NS = N // NC                    # nodes per core
        self.NW = math.ceil(self.NS / P)     # src windows per core
        self.CHUNK = CHUNK                   # dst chunk (int16 idx range)
        self.NCH = math.ceil(N / CHUNK)      # dst chunks
        self.QUOTA = QUOTA                   # edge slots per (window, chunk)
        self.CT = QUOTA // P                 # tiles per cell
        self.GW = GW                         # windows per gather group
        self.groups = [
            list(range(i, min(i + GW, self.NW)))
            for i in range(0, self.NW, GW)
        ]
        self.TT = self.NW * self.NCH * self.CT   # total tiles per core
        self.ROW = 256                       # table row elems (bf16)


CFG = Cfg()


def _mk_nc(num_devices):
    return bacc.Bacc(
        "TRN2",
        target_bir_lowering=False,
        debug=False,
        enable_asserts=True,
        num_devices=num_devices,
    )


# ---------------------------------------------------------------- L1: stats
def build_l1(cfg):
    nc = _mk_nc(cfg.NC)
    x = nc.dram_tensor("x_slice", [cfg.NS, cfg.D], F32, kind="ExternalInput")
    stats = nc.dram_tensor("stats", [cfg.D, 2], F32, kind="ExternalOutput")
    ntiles = math.ceil(cfg.NS / P)
    with tile.TileContext(nc) as tc:
        with (
            tc.tile_pool(name="sb", bufs=4) as sb,
            tc.tile_pool(name="cst", bufs=1) as cst,
            tc.tile_pool(name="ps", bufs=1, space="PSUM") as ps,
        ):
            ones = cst.tile([P, 1], F32)
            nc.gpsimd.memset(ones[:], 1.0)
            acc0 = ps.tile([cfg.D, 1], F32, tag="a0")
            acc1 = ps.tile([cfg.D, 1], F32, tag="a1")
            for t in range(ntiles):
                r0 = t * P
                rows = min(P, cfg.NS - r0)
                xt = sb.tile([P, cfg.D], F32, tag="xt")
                nc.sync.dma_start(xt[:rows, :], x[r0 : r0 + rows, :])
                xsq = sb.tile([P, cfg.D], F32, tag="xsq")
                nc.vector.tensor_tensor(
                    out=xsq[:rows, :], in0=xt[:rows, :], in1=xt[:rows, :], op=OP.mult
                )
                nc.tensor.matmul(
                    acc0[:, 0:1], xt[:rows, :], ones[:rows, :],
                    start=(t == 0), stop=(t == ntiles - 1),
                )
                nc.tensor.matmul(
                    acc1[:, 0:1], xsq[:rows, :], ones[:rows, :],
                    start=(t == 0), stop=(t == ntiles - 1),
                )
            out_sb = cst.tile([cfg.D, 2], F32, tag="o")
            nc.vector.tensor_copy(out_sb[:, 0:1], acc0[:])
            nc.vector.tensor_copy(out_sb[:, 1:2], acc1[:])
            nc.sync.dma_start(stats[:], out_sb[:])
    nc.compile()
    return nc


# ------------------------------------------------------------ L2: node phase
def build_l2(cfg):
    nc = _mk_nc(cfg.NC)
    D, NS, NW = cfg.D, cfg.NS, cfg.NW
    xT = nc.dram_tensor("xT_slice", [D, NS], F32, kind="ExternalInput")
    scale = nc.dram_tensor("scale", [D, 1], F32, kind="ExternalInput")
    shift = nc.dram_tensor("shift", [D, 1], F32, kind="ExternalInput")
    k0 = nc.dram_tensor("k0", [D, D], F32, kind="ExternalInput")
    k1 = nc.dram_tensor("k1", [D, D], F32, kind="ExternalInput")
    k2 = nc.dram_tensor("k2", [D, D], F32, kind="ExternalInput")
    mappedT = nc.dram_tensor("mappedT", [D, NS], F32, kind="ExternalOutput")
    s1o = nc.dram_tensor("s1o", [P, NW], F32, kind="ExternalOutput")
    s2o = nc.dram_tensor("s2o", [P, NW], F32, kind="ExternalOutput")

    with tile.TileContext(nc) as tc:
        with (
            tc.tile_pool(name="cst", bufs=1) as cst,
            tc.tile_pool(name="sb", bufs=4) as sb,
            tc.tile_pool(name="ps", bufs=4, space="PSUM") as ps,
            tc.tile_pool(name="ps1", bufs=2, space="PSUM") as ps1,
        ):
            ksb = cst.tile([D, D], F32, tag="k0")
            k1sb = cst.tile([D, D], F32, tag="k1")
            k2sb = cst.tile([D, D], F32, tag="k2")
            ssb = cst.tile([D, 1], F32, tag="sc")
            bsb = cst.tile([D, 1], F32, tag="sh")
            ones = cst.tile([D, 1], F32, tag="on")
            s1sb = cst.tile([P, NW], F32, tag="s1")
            s2sb = cst.tile([P, NW], F32, tag="s2")
            nc.sync.dma_start(ksb[:], k0[:])
            nc.sync.dma_start(k1sb[:], k1[:])
            nc.sync.dma_start(k2sb[:], k2[:])
            nc.sync.dma_start(ssb[:], scale[:])
            nc.sync.dma_start(bsb[:], shift[:])
            nc.gpsimd.memset(ones[:], 1.0)
            nc.gpsimd.memset(s1sb[:], 0.0)
            nc.gpsimd.memset(s2sb[:], 0.0)

            for t in range(NW):
                c0 = t * P
                cols = min(P, NS - c0)
                xt = sb.tile([D, P], F32, tag="xt")
                nc.sync.dma_start(xt[:, :cols], xT[:, c0 : c0 + cols])
                xn = sb.tile([D, P], F32, tag="xn")
                nc.scalar.activation(
                    out=xn[:, :cols], in_=xt[:, :cols], func=AF.Identity,
                    bias=bsb[:, 0:1], scale=ssb[:, 0:1],
                )
                mps = ps.tile([D, P], F32, tag="mm")
                nc.tensor.matmul(mps[:, :cols], ksb[:], xn[:, :cols],
                                 start=True, stop=True)
                msb = sb.tile([D, P], F32, tag="ms")
                nc.scalar.copy(out=msb[:, :cols], in_=mps[:, :cols])
                nc.sync.dma_start(mappedT[:, c0 : c0 + cols], msb[:, :cols])
                for (kw, ssl) in ((k1sb, s1sb), (k2sb, s2sb)):
                    yps = ps.tile([D, P], F32, tag="mm")
                    nc.tensor.matmul(yps[:, :cols], kw[:], xn[:, :cols],
                                     start=True, stop=True)
                    z = sb.tile([D, P], F32, tag="z")
                    nc.vector.tensor_tensor(
                        out=z[:, :cols], in0=yps[:, :cols], in1=xn[:, :cols],
                        op=OP.mult,
                    )
                    sps = ps1.tile([P, 1], F32, tag="s")
                    nc.tensor.matmul(sps[:cols, :], z[:, :cols], ones[:],
                                     start=True, stop=True)
                    nc.scalar.activation(
                        out=ssl[:cols, t : t + 1], in_=sps[:cols, :], func=AF.Tanh
                    )
            nc.sync.dma_start(s1o[:], s1sb[:])
            nc.sync.dma_start(s2o[:], s2sb[:])
    nc.compile()
    return nc


# ------------------------------------------------------------ L3: edge phase
def build_l3(cfg):
    nc = _mk_nc(cfg.NC)
    NS, NW, NCH, CT, ROW = cfg.NS, cfg.NW, cfg.NCH, cfg.CT, cfg.ROW
    TT = cfg.TT
    IC = TT * P // 16  # idx cols (int16, 16-wrap)

    table = nc.dram_tensor("table", [cfg.N, ROW], BF16, kind="ExternalInput")
    idxs_d = nc.dram_tensor("idxs", [P, IC], I16, kind="ExternalInput")
    srel_d = nc.dram_tensor("srel", [P, TT], F32, kind="ExternalInput")
    s1r_d = nc.dram_tensor("s1rows", [NW, P], F32, kind="ExternalInput")
    out_d = nc.dram_tensor("out", [NS, 130], F32, kind="ExternalOutput")

    iota_np = np.broadcast_to(
        np.arange(P, dtype=np.float32), (P, P)
    ).astype(ml_dtypes.bfloat16)
    iota_dram = nc.inline_tensor(np.ascontiguousarray(iota_np), name="iota_c")
    ones1p_dram = nc.inline_tensor(np.ones((1, P), np.float32), name="ones1p_c")

    with tile.TileContext(nc) as tc:
        with (
            tc.tile_pool(name="cst", bufs=1) as cst,
            tc.tile_pool(name="gb", bufs=2) as gbp,
            tc.tile_pool(name="s01", bufs=2) as s01p,
            tc.tile_pool(name="wb", bufs=2) as wbp,
            tc.tile_pool(name="jk", bufs=4) as jkp,
            tc.tile_pool(name="sw", bufs=4) as swp,
            tc.tile_pool(name="sm", bufs=4) as smp,
            tc.tile_pool(name="rp", bufs=2) as rpp,
            tc.tile_pool(name="acc", bufs=1, space="PSUM") as accp,
            tc.tile_pool(name="rps", bufs=2, space="PSUM") as rpsp,
        ):
            iota_bf = cst.tile([P, P], BF16, tag="iota")
            nc.sync.dma_start(iota_bf[:], iota_dram.ap())
            ones1p = cst.tile([1, P], F32, tag="ones1p")
            nc.sync.dma_start(ones1p[:], ones1p_dram.ap())
            idx_sb = cst.tile([P, IC], I16, tag="idx")
            nc.sync.dma_start(idx_sb[:], idxs_d[:])
            srel_sb = cst.tile([P, TT], F32, tag="srel")
            nc.sync.dma_start(srel_sb[:], srel_d[:])

            gt0 = 0     # global tile base of current group
            ic0 = 0     # global idx col base
            for g in cfg.groups:
                L = len(g)
                ntile = L * NCH * CT           # tiles in this group
                gbuf = gbp.tile([P, ntile, ROW], BF16, tag="gbuf")
                for c in range(NCH):
                    n_idx = L * cfg.QUOTA
                    nc.gpsimd.dma_gather(
                        out_ap=gbuf[:, c * L * CT : (c + 1) * L * CT, :],
                        in_ap=table[
                            c * cfg.CHUNK : min((c + 1) * cfg.CHUNK, cfg.N), :
                        ],
                        idxs_ap=idx_sb[:, ic0 : ic0 + n_idx // 16],
                        num_idxs=n_idx,
                        num_idxs_reg=n_idx,
                        elem_size=ROW,
                        single_packet=False,
                    )
                    ic0 += n_idx // 16

                # replicate s1 rows of this group's windows across partitions
                repl = rpp.tile([P, L, P], BF16, tag="repl")
                for wi, w in enumerate(g):
                    s1row = smp.tile([1, P], F32, tag="s1row")
                    nc.sync.dma_start(s1row[:], s1r_d[w : w + 1, :])
                    rps = rpsp.tile([P, P], F32, tag="rps")
                    nc.tensor.matmul(rps[:], ones1p[:], s1row[:],
                                     start=True, stop=True)
                    nc.scalar.copy(out=repl[:, wi, :], in_=rps[:])

                # one-hots + per-edge s1 (tbat) per tile
                s01g = s01p.tile([P, ntile, P], BF16, tag="s01")
                tbat = wbp.tile([P, ntile, 1], F32, tag="tbat")
                for t in range(ntile):
                    wi = (t % (L * CT)) // CT
                    nc.vector.tensor_scalar(
                        out=s01g[:, t, :],
                        in0=iota_bf[:],
                        scalar1=srel_sb[:, gt0 + t : gt0 + t + 1],
                        scalar2=None,
                        op0=OP.is_equal,
                    )
                    junk = jkp.tile([P, P], BF16, tag="junk")
                    nc.vector.scalar_tensor_tensor(
                        out=junk[:],
                        in0=s01g[:, t, :],
                        scalar=1.0,
                        in1=repl[:, wi, :],
                        op0=OP.mult,
                        op1=OP.mult,
                        accum_out=tbat[:, t, :],
                    )

                # w = exp(leaky_relu(s1 + s2)) for the whole group
                s2f = wbp.tile([P, ntile, 1], F32, tag="s2f")
                nc.vector.tensor_copy(out=s2f[:], in_=gbuf[:, :, 128:129])
                wq = wbp.tile([P, ntile, 1], F32, tag="wq")
                nc.vector.tensor_tensor(out=wq[:], in0=tbat[:], in1=s2f[:],
                                        op=OP.add)
                nc.vector.scalar_tensor_tensor(
                    out=wq[:], in0=wq[:], scalar=0.01, in1=wq[:],
                    op0=OP.mult, op1=OP.max,
                )
                wf = wbp.tile([P, ntile, 1], F32, tag="wf")
                nc.scalar.activation(out=wf[:], in_=wq[:], func=AF.Exp)

                # scale one-hots by w; ONE matmul per tile: num cols 0:128,
                # (dead col 128), den col 129 (table ones column)
                accs = {}
                for wi, w in enumerate(g):
                    accs[wi] = accp.tile([P, 130], F32, tag=f"acc{wi}",
                                         name=f"acc{wi}")
                for t in range(ntile):
                    c = t // (L * CT)
                    wi = (t % (L * CT)) // CT
                    k = t % CT
                    s01w = swp.tile([P, P], BF16, tag="s01w")
                    nc.scalar.activation(
                        out=s01w[:], in_=s01g[:, t, :], func=AF.Identity,
                        scale=wf[:, t, :],
                    )
                    nc.tensor.matmul(
                        accs[wi][:, 0:130], s01w[:], gbuf[:, t, 0:130],
                        start=(c == 0 and k == 0),
                        stop=(c == NCH - 1 and k == CT - 1),
                    )

                # finalize windows: write raw [num | s2dead | den]
                for wi, w in enumerate(g):
                    rows = min(P, NS - w * P)
                    osb = smp.tile([P, 130], F32, tag="osb")
                    nc.scalar.copy(out=osb[:rows, :], in_=accs[wi][:rows, :])
                    nc.sync.dma_start(out_d[w * P : w * P + rows, :],
                                      osb[:rows, :])
                gt0 += ntile
    nc.compile()
    return nc


# ------------------------------------------------------------ host planning
def plan_edges(edge_index, cfg):
    """Bucket edges into fixed QUOTA-slot (window, chunk) cells per core.

    Returns per-core streams {idxs, srel} and the spilled edge arrays."""
    src = np.asarray(edge_index[0], dtype=np.int64)
    dst = np.asarray(edge_index[1], dtype=np.int64)
    NC, NS, NW, NCH, Q = cfg.NC, cfg.NS, cfg.NW, cfg.NCH, cfg.QUOTA
    CH, CT = cfg.CHUNK, cfg.CT
    owner = src // NS
    w = (src % NS) // P
    srel_v = (src % NS) % P
    ch = dst // CH
    key = (owner * NW + w) * NCH + ch
    order = np.argsort(key, kind="stable")
    key_s = key[order]
    bounds = np.searchsorted(key_s, np.arange(NC * NW * NCH + 1))

    # slot order within a core: for g in groups: for c: for w in g: Q slots
    cell_slot = np.empty((NW, NCH), np.int64)
    pos = 0
    for g in cfg.groups:
        for c in range(NCH):
            for ww in g:
                cell_slot[ww, c] = pos
                pos += Q
    nslot = pos
    assert nslot == cfg.TT * P

    streams = []
    spill_parts = []
    for core in range(NC):
        idx_arr = np.zeros(nslot, np.int16)
        srel_arr = np.full(nslot, 200.0, np.float32)
        for ww in range(NW):
            for c in range(NCH):
                b = (core * NW + ww) * NCH + c
                lo, hi = bounds[b], bounds[b + 1]
                take = min(Q, hi - lo)
                sel = order[lo : lo + take]
                base = cell_slot[ww, c]
                idx_arr[base : base + take] = (dst[sel] - c * CH).astype(
                    np.int16)
                srel_arr[base : base + take] = srel_v[sel].astype(np.float32)
                if hi - lo > Q:
                    spill_parts.append(order[lo + Q : hi])
        # wrap idx per gather call (call = L*Q consecutive slots)
        blocks = []
        s0 = 0
        for g in cfg.groups:
            L = len(g)
            for c in range(NCH):
                n = L * Q
                blk = idx_arr[s0 : s0 + n]
                blocks.append(np.tile(blk.reshape(-1, 16).T, (8, 1)))
                s0 += n
        idxs = np.ascontiguousarray(np.concatenate(blocks, axis=1))
        srel_T = np.ascontiguousarray(srel_arr.reshape(-1, P).T)
        streams.append({"idxs": idxs, "srel": srel_T})
    spill = (np.concatenate(spill_parts) if spill_parts
             else np.zeros(0, np.int64))
    return streams, src[spill], dst[spill]


# ------------------------------------------------------------ orchestration
def _run(nc, in_maps, cfg, **kw):
    if RUN_MODE == "sim":
        from concourse.bass_interp import MultiCoreSim

        sim = MultiCoreSim(nc, num_cores=cfg.NC, trace=False)
        for ci, core in enumerate(sim.cores.values()):
            for name, arr in in_maps[ci].items():
                core.tensor(name)[:] = arr
        sim.simulate(check_with_hw=False)
        out_names = []
        for alloc in nc.m.functions[0].allocations:
            if not isinstance(alloc, mybir.MemoryLocationSet):
                continue
            if alloc.kind == "ExternalOutput":
                out_names.append(alloc.memorylocations[0].name)
        results = [
            {n: np.array(core.tensor(n)) for n in out_names}
            for core in sim.cores.values()
        ]

        class R:
            pass

        r = R()
        r.results = results
        r.exec_time_ns = None
        return r
    return bass_utils.run_bass_kernel_spmd(
        nc, in_maps, core_ids=list(range(cfg.NC)), **kw
    )


def kernel(x, edge_index, kernel, kernel1, kernel2, gamma, beta, _cfg=None,
           _trace=False):
    cfg = _cfg or CFG
    x = np.asarray(x, np.float32)
    k0 = np.asarray(kernel, np.float32)
    k1 = np.asarray(kernel1, np.float32)
    k2 = np.asarray(kernel2, np.float32)
    gamma = np.asarray(gamma, np.float32)
    beta = np.asarray(beta, np.float32)
    NC, NS, D = cfg.NC, cfg.NS, cfg.D

    import time as _t
    _lap_t = [_t.time()]

    def _lap(msg):
        now = _t.time()
        print(f"[kernel] {msg}: +{now - _lap_t[0]:.1f}s", flush=True)
        _lap_t[0] = now

    # ---- L1
    nc1 = build_l1(cfg)
    _lap("build_l1")
    in1 = [{"x_slice": np.ascontiguousarray(x[c * NS : (c + 1) * NS])}
           for c in range(NC)]
    r1 = _run(nc1, in1, cfg, trace=_trace)
    _lap("run_l1")
    parts = np.stack([r1.results[c]["stats"] for c in range(NC)])
    tot = parts.sum(axis=0).astype(np.float64)
    mean = tot[:, 0] / cfg.N
    var = tot[:, 1] / cfg.N - mean * mean
    rstd = gamma.astype(np.float64) / np.sqrt(var + BN_EPS)
    scale = rstd.astype(np.float32)
    shift = (beta.astype(np.float64) - mean * rstd).astype(np.float32)

    # ---- L2
    nc2 = build_l2(cfg)
    _lap("build_l2")
    in2 = []
    for c in range(NC):
        in2.append({
            "xT_slice": np.ascontiguousarray(x[c * NS : (c + 1) * NS].T),
            "scale": np.ascontiguousarray(scale.reshape(D, 1)),
            "shift": np.ascontiguousarray(shift.reshape(D, 1)),
            "k0": k0, "k1": k1, "k2": k2,
        })
    r2 = _run(nc2, in2, cfg, trace=_trace)
    _lap("run_l2")
    mapped = np.concatenate(
        [np.asarray(r2.results[c]["mappedT"]).T for c in range(NC)], axis=0
    )
    s1 = np.concatenate(
        [np.asarray(r2.results[c]["s1o"]).T.reshape(-1)[:NS] for c in range(NC)]
    )
    s2 = np.concatenate(
        [np.asarray(r2.results[c]["s2o"]).T.reshape(-1)[:NS] for c in range(NC)]
    )

    # ---- host glue: table + edge streams
    tbl = np.zeros((cfg.N, cfg.ROW), ml_dtypes.bfloat16)
    tbl[:, 0:128] = mapped.astype(ml_dtypes.bfloat16)
    tbl[:, 128] = s2.astype(ml_dtypes.bfloat16)
    tbl[:, 129] = 1.0
    streams, sp_src, sp_dst = plan_edges(edge_index, cfg)
    _lap(f"host_glue (spill={len(sp_src)})")

    # ---- L3
    nc3 = build_l3(cfg)
    _lap("build_l3")
    in3 = []
    for c in range(NC):
        s1pad = np.zeros(cfg.NW * P, np.float32)
        s1pad[:NS] = s1[c * NS : (c + 1) * NS]
        in3.append({
            "table": tbl,
            "idxs": streams[c]["idxs"],
            "srel": streams[c]["srel"],
            "s1rows": np.ascontiguousarray(s1pad.reshape(cfg.NW, P)),
        })
    r3 = _run(nc3, in3, cfg, trace=_trace)
    _lap("run_l3")
    raw = np.concatenate(
        [np.asarray(r3.results[c]["out"]) for c in range(NC)], axis=0
    )
    num = raw[:, 0:128].astype(np.float64)
    den = raw[:, 129].astype(np.float64)

    # ---- spill edges on host
    if len(sp_src):
        e = s1[sp_src] + s2[sp_dst]
        el = np.where(e > 0, e, 0.01 * e)
        wsp = np.exp(el).astype(np.float64)
        mb = tbl[:, 0:128].astype(np.float32).astype(np.float64)
        np.add.at(num, sp_src, wsp[:, None] * mb[sp_dst])
        np.add.at(den, sp_src, wsp)

    out = np.maximum(num / np.maximum(den, 1e-16)[:, None], 0.0)
    globals()["_LAST_RESULTS"] = (r1, r2, r3)
    return out.astype(np.float32)


# revision 6
# speedup vs baseline: 1.3856x; 1.3234x over previous
"""AliNet graph-attention layer on 8 Trainium2 NeuronCores (v2).

Pipeline (3 SPMD launches; host does sharding glue + spill edges):
  L1: per-core BN partial sums over its node slice  -> host combines stats
  L2: per-core node phase: xn = BN(x), mapped = xn@K, s1/s2 = tanh(rowdot)
  host: packs table[N, 256] bf16 rows: cols 0:128 mapped, 128 s2, 129 ones;
        buckets edges into fixed 512-slot (window, chunk) cells (overflow
        edges spill to host numpy), builds gather idx + srel streams
  L3: per-core edge phase over its src-owned edges:
        dma_gather rows by dst (row-rate-bound: bytes are free),
        one-hot by srel on DVE, per-edge s1 via replicate+reduce,
        w = exp(leaky_relu(s1+s2)) (Scalar exp), scale one-hot by w,
        ONE matmul per tile accumulates num (cols 0:128) AND den (col 129)
        in PSUM; outputs raw [num|den] per node
  host: adds spill contributions, out = relu(num / max(den, 1e-16))
"""

import math
import numpy as np
import ml_dtypes

import concourse.bass as bass
import concourse.bacc as bacc
import concourse.tile as tile
import concourse.mybir as mybir
import concourse.bass_utils as bass_utils

F32 = mybir.dt.float32
BF16 = mybir.dt.bfloat16
I16 = mybir.dt.int16
I32 = mybir.dt.int32
AF = mybir.ActivationFunctionType
OP = mybir.AluOpType

BN_EPS = 1e-5
P = 128

RUN_MODE = "hw"  # "hw" or "sim"


class Cfg:
    def __init__(self, N=100000, D=128, NC=8, CHUNK=25000, QUOTA=512, GW=4):
        self.N, self.D, self.NC = N, D, NC
        assert N % NC == 0
        self.NS = N // NC                    # nodes per core
        self.NW = math.ceil(self.NS / P)     # src windows per core
        self.CHUNK = CHUNK                   # dst chunk (int16 idx range)
        self.NCH = math.ceil(N / CHUNK)      # dst chunks
        self.QUOTA = QUOTA                   # edge slots per (window, chunk)
        self.CT = QUOTA // P                 # tiles per cell
        self.GW = GW                         # windows per gather group
        self.groups = [
            list(range(i, min(i + GW, self.NW)))
            for i in range(0, self.NW, GW)
        ]
        self.TT = self.NW * self.NCH * self.CT   # total tiles per core
        self.ROW = 256                       # table row elems (bf16)


CFG = Cfg()


def _mk_nc(num_devices):
    return bacc.Bacc(
        "TRN2",
        target_bir_lowering=False,
        debug=False,
        enable_asserts=True,
        num_devices=num_devices,
    )


# ---------------------------------------------------------------- L1: stats
def build_l1(cfg):
    nc = _mk_nc(cfg.NC)
    x = nc.dram_tensor("x_slice", [cfg.NS, cfg.D], F32, kind="ExternalInput")
    stats = nc.dram_tensor("stats", [cfg.D, 2], F32, kind="ExternalOutput")
    ntiles = math.ceil(cfg.NS / P)
    with tile.TileContext(nc) as tc:
        with (
            tc.tile_pool(name="sb", bufs=4) as sb,
            tc.tile_pool(name="cst", bufs=1) as cst,
            tc.tile_pool(name="ps", bufs=1, space="PSUM") as ps,
        ):
            ones = cst.tile([P, 1], F32)
            nc.gpsimd.memset(ones[:], 1.0)
            acc0 = ps.tile([cfg.D, 1], F32, tag="a0")
            acc1 = ps.tile([cfg.D, 1], F32, tag="a1")
            for t in range(ntiles):
                r0 = t * P
                rows = min(P, cfg.NS - r0)
                xt = sb.tile([P, cfg.D], F32, tag="xt")
                nc.sync.dma_start(xt[:rows, :], x[r0 : r0 + rows, :])
                xsq = sb.tile([P, cfg.D], F32, tag="xsq")
                nc.vector.tensor_tensor(
                    out=xsq[:rows, :], in0=xt[:rows, :], in1=xt[:rows, :], op=OP.mult
                )
                nc.tensor.matmul(
                    acc0[:, 0:1], xt[:rows, :], ones[:rows, :],
                    start=(t == 0), stop=(t == ntiles - 1),
                )
                nc.tensor.matmul(
                    acc1[:, 0:1], xsq[:rows, :], ones[:rows, :],
                    start=(t == 0), stop=(t == ntiles - 1),
                )
            out_sb = cst.tile([cfg.D, 2], F32, tag="o")
            nc.vector.tensor_copy(out_sb[:, 0:1], acc0[:])
            nc.vector.tensor_copy(out_sb[:, 1:2], acc1[:])
            nc.sync.dma_start(stats[:], out_sb[:])
    nc.compile()
    return nc


# ------------------------------------------------------------ L2: node phase
def build_l2(cfg):
    nc = _mk_nc(cfg.NC)
    D, NS, NW = cfg.D, cfg.NS, cfg.NW
    xT = nc.dram_tensor("xT_slice", [D, NS], F32, kind="ExternalInput")
    scale = nc.dram_tensor("scale", [D, 1], F32, kind="ExternalInput")
    shift = nc.dram_tensor("shift", [D, 1], F32, kind="ExternalInput")
    k0 = nc.dram_tensor("k0", [D, D], F32, kind="ExternalInput")
    k1 = nc.dram_tensor("k1", [D, D], F32, kind="ExternalInput")
    k2 = nc.dram_tensor("k2", [D, D], F32, kind="ExternalInput")
    mappedT = nc.dram_tensor("mappedT", [D, NS], F32, kind="ExternalOutput")
    s1o = nc.dram_tensor("s1o", [P, NW], F32, kind="ExternalOutput")
    s2o = nc.dram_tensor("s2o", [P, NW], F32, kind="ExternalOutput")

    with tile.TileContext(nc) as tc:
        with (
            tc.tile_pool(name="cst", bufs=1) as cst,
            tc.tile_pool(name="sb", bufs=4) as sb,
            tc.tile_pool(name="ps", bufs=4, space="PSUM") as ps,
            tc.tile_pool(name="ps1", bufs=2, space="PSUM") as ps1,
        ):
            ksb = cst.tile([D, D], F32, tag="k0")
            k1sb = cst.tile([D, D], F32, tag="k1")
            k2sb = cst.tile([D, D], F32, tag="k2")
            ssb = cst.tile([D, 1], F32, tag="sc")
            bsb = cst.tile([D, 1], F32, tag="sh")
            ones = cst.tile([D, 1], F32, tag="on")
            s1sb = cst.tile([P, NW], F32, tag="s1")
            s2sb = cst.tile([P, NW], F32, tag="s2")
            nc.sync.dma_start(ksb[:], k0[:])
            nc.sync.dma_start(k1sb[:], k1[:])
            nc.sync.dma_start(k2sb[:], k2[:])
            nc.sync.dma_start(ssb[:], scale[:])
            nc.sync.dma_start(bsb[:], shift[:])
            nc.gpsimd.memset(ones[:], 1.0)
            nc.gpsimd.memset(s1sb[:], 0.0)
            nc.gpsimd.memset(s2sb[:], 0.0)

            for t in range(NW):
                c0 = t * P
                cols = min(P, NS - c0)
                xt = sb.tile([D, P], F32, tag="xt")
                nc.sync.dma_start(xt[:, :cols], xT[:, c0 : c0 + cols])
                xn = sb.tile([D, P], F32, tag="xn")
                nc.scalar.activation(
                    out=xn[:, :cols], in_=xt[:, :cols], func=AF.Identity,
                    bias=bsb[:, 0:1], scale=ssb[:, 0:1],
                )
                mps = ps.tile([D, P], F32, tag="mm")
                nc.tensor.matmul(mps[:, :cols], ksb[:], xn[:, :cols],
                                 start=True, stop=True)
                msb = sb.tile([D, P], F32, tag="ms")
                nc.scalar.copy(out=msb[:, :cols], in_=mps[:, :cols])
                nc.sync.dma_start(mappedT[:, c0 : c0 + cols], msb[:, :cols])
                for (kw, ssl) in ((k1sb, s1sb), (k2sb, s2sb)):
                    yps = ps.tile([D, P], F32, tag="mm")
                    nc.tensor.matmul(yps[:, :cols], kw[:], xn[:, :cols],
                                     start=True, stop=True)
                    z = sb.tile([D, P], F32, tag="z")
                    nc.vector.tensor_tensor(
                        out=z[:, :cols], in0=yps[:, :cols], in1=xn[:, :cols],
                        op=OP.mult,
                    )
                    sps = ps1.tile([P, 1], F32, tag="s")
                    nc.tensor.matmul(sps[:cols, :], z[:, :cols], ones[:],
                                     start=True, stop=True)
                    nc.scalar.activation(
                        out=ssl[:cols, t : t + 1], in_=sps[:cols, :], func=AF.Tanh
                    )
            nc.sync.dma_start(s1o[:], s1sb[:])
            nc.sync.dma_start(s2o[:], s2sb[:])
    nc.compile()
    return nc


# ------------------------------------------------------------ L3: edge phase
def build_l3(cfg):
    nc = _mk_nc(cfg.NC)
    NS, NW, NCH, CT, ROW = cfg.NS, cfg.NW, cfg.NCH, cfg.CT, cfg.ROW
    TT = cfg.TT
    IC = TT * P // 16  # idx cols (int16, 16-wrap)

    table = nc.dram_tensor("table", [cfg.N, ROW], BF16, kind="ExternalInput")
    idxs_d = nc.dram_tensor("idxs", [P, IC], I16, kind="ExternalInput")
    srel_d = nc.dram_tensor("srel", [P, TT], F32, kind="ExternalInput")
    s1r_d = nc.dram_tensor("s1rows", [NW, P], F32, kind="ExternalInput")
    out_d = nc.dram_tensor("out", [NS, 130], F32, kind="ExternalOutput")

    iota_np = np.broadcast_to(
        np.arange(P, dtype=np.float32), (P, P)
    ).astype(ml_dtypes.bfloat16)
    iota_dram = nc.inline_tensor(np.ascontiguousarray(iota_np), name="iota_c")
    ones1p_dram = nc.inline_tensor(np.ones((1, P), np.float32), name="ones1p_c")

    with tile.TileContext(nc) as tc:
        with (
            tc.tile_pool(name="cst", bufs=1) as cst,
            tc.tile_pool(name="gb", bufs=2) as gbp,
            tc.tile_pool(name="s01", bufs=2) as s01p,
            tc.tile_pool(name="wb", bufs=2) as wbp,
            tc.tile_pool(name="jk", bufs=4) as jkp,
            tc.tile_pool(name="sw", bufs=4) as swp,
            tc.tile_pool(name="sm", bufs=4) as smp,
            tc.tile_pool(name="rp", bufs=2) as rpp,
            tc.tile_pool(name="acc", bufs=1, space="PSUM") as accp,
            tc.tile_pool(name="rps", bufs=2, space="PSUM") as rpsp,
        ):
            iota_bf = cst.tile([P, P], BF16, tag="iota")
            nc.sync.dma_start(iota_bf[:], iota_dram.ap())
            ones1p = cst.tile([1, P], F32, tag="ones1p")
            nc.sync.dma_start(ones1p[:], ones1p_dram.ap())
            idx_sb = cst.tile([P, IC], I16, tag="idx")
            nc.sync.dma_start(idx_sb[:], idxs_d[:])
            srel_sb = cst.tile([P, TT], F32, tag="srel")
            nc.sync.dma_start(srel_sb[:], srel_d[:])

            gt0 = 0     # global tile base of current group
            ic0 = 0     # global idx col base
            for g in cfg.groups:
                L = len(g)
                ntile = L * NCH * CT           # tiles in this group
                gbuf = gbp.tile([P, ntile, ROW], BF16, tag="gbuf")
                for c in range(NCH):
                    n_idx = L * cfg.QUOTA
                    nc.gpsimd.dma_gather(
                        out_ap=gbuf[:, c * L * CT : (c + 1) * L * CT, :],
                        in_ap=table[
                            c * cfg.CHUNK : min((c + 1) * cfg.CHUNK, cfg.N), :
                        ],
                        idxs_ap=idx_sb[:, ic0 : ic0 + n_idx // 16],
                        num_idxs=n_idx,
                        num_idxs_reg=n_idx,
                        elem_size=ROW,
                        single_packet=False,
                    )
                    ic0 += n_idx // 16

                # replicate s1 rows of this group's windows across partitions
                repl = rpp.tile([P, L, P], BF16, tag="repl")
                for wi, w in enumerate(g):
                    s1row = smp.tile([1, P], F32, tag="s1row")
                    nc.sync.dma_start(s1row[:], s1r_d[w : w + 1, :])
                    rps = rpsp.tile([P, P], F32, tag="rps")
                    nc.tensor.matmul(rps[:], ones1p[:], s1row[:],
                                     start=True, stop=True)
                    nc.scalar.copy(out=repl[:, wi, :], in_=rps[:])

                # one-hots + per-edge s1 (tbat) per tile — no gather deps,
                # issued first so the DVE never idles behind gather waits
                s01g = s01p.tile([P, ntile, P], BF16, tag="s01")
                tbat = wbp.tile([P, ntile, 1], F32, tag="tbat")
                for t in range(ntile):
                    wi = (t % (L * CT)) // CT
                    nc.vector.tensor_scalar(
                        out=s01g[:, t, :],
                        in0=iota_bf[:],
                        scalar1=srel_sb[:, gt0 + t : gt0 + t + 1],
                        scalar2=None,
                        op0=OP.is_equal,
                    )
                    junk = jkp.tile([P, P], BF16, tag="junk")
                    nc.vector.scalar_tensor_tensor(
                        out=junk[:],
                        in0=s01g[:, t, :],
                        scalar=1.0,
                        in1=repl[:, wi, :],
                        op0=OP.mult,
                        op1=OP.mult,
                        accum_out=tbat[:, t, :],
                    )

                # per chunk-section: w-chain depends only on that section's
                # gather call, so compute overlaps the remaining calls
                accs = {}
                for wi, w in enumerate(g):
                    accs[wi] = accp.tile([P, 130], F32, tag=f"acc{wi}",
                                         name=f"acc{wi}")
                sec = L * CT
                for c in range(NCH):
                    t0 = c * sec
                    s2f = wbp.tile([P, sec, 1], F32, tag="s2f")
                    nc.scalar.copy(out=s2f[:],
                                   in_=gbuf[:, t0 : t0 + sec, 128:129])
                    wq = wbp.tile([P, sec, 1], F32, tag="wq")
                    nc.vector.tensor_tensor(
                        out=wq[:], in0=tbat[:, t0 : t0 + sec, :], in1=s2f[:],
                        op=OP.add)
                    nc.vector.scalar_tensor_tensor(
                        out=wq[:], in0=wq[:], scalar=0.01, in1=wq[:],
                        op0=OP.mult, op1=OP.max,
                    )
                    wf = wbp.tile([P, sec, 1], F32, tag="wf")
                    nc.scalar.activation(out=wf[:], in_=wq[:], func=AF.Exp)

                    # scale one-hots by w; ONE matmul per tile: num cols
                    # 0:128, (dead col 128), den col 129 (table ones column)
                    for ts in range(sec):
                        t = t0 + ts
                        wi = ts // CT
                        k = ts % CT
                        s01w = swp.tile([P, P], BF16, tag="s01w")
                        nc.scalar.activation(
                            out=s01w[:], in_=s01g[:, t, :], func=AF.Identity,
                            scale=wf[:, ts, :],
                        )
                        nc.tensor.matmul(
                            accs[wi][:, 0:130], s01w[:], gbuf[:, t, 0:130],
                            start=(c == 0 and k == 0),
                            stop=(c == NCH - 1 and k == CT - 1),
                        )

                # finalize windows: write raw [num | s2dead | den]
                for wi, w in enumerate(g):
                    rows = min(P, NS - w * P)
                    osb = smp.tile([P, 130], F32, tag="osb")
                    nc.scalar.copy(out=osb[:rows, :], in_=accs[wi][:rows, :])
                    nc.sync.dma_start(out_d[w * P : w * P + rows, :],
                                      osb[:rows, :])
                gt0 += ntile
    nc.compile()
    return nc


# ------------------------------------------------------------ host planning
def plan_edges(edge_index, cfg):
    """Bucket edges into fixed QUOTA-slot (window, chunk) cells per core.

    Returns per-core streams {idxs, srel} and the spilled edge arrays."""
    src = np.asarray(edge_index[0], dtype=np.int64)
    dst = np.asarray(edge_index[1], dtype=np.int64)
    NC, NS, NW, NCH, Q = cfg.NC, cfg.NS, cfg.NW, cfg.NCH, cfg.QUOTA
    CH, CT = cfg.CHUNK, cfg.CT
    owner = src // NS
    w = (src % NS) // P
    srel_v = (src % NS) % P
    ch = dst // CH
    key = (owner * NW + w) * NCH + ch
    order = np.argsort(key, kind="stable")
    key_s = key[order]
    bounds = np.searchsorted(key_s, np.arange(NC * NW * NCH + 1))

    # slot order within a core: for g in groups: for c: for w in g: Q slots
    cell_slot = np.empty((NW, NCH), np.int64)
    pos = 0
    for g in cfg.groups:
        for c in range(NCH):
            for ww in g:
                cell_slot[ww, c] = pos
                pos += Q
    nslot = pos
    assert nslot == cfg.TT * P

    streams = []
    spill_parts = []
    for core in range(NC):
        idx_arr = np.zeros(nslot, np.int16)
        srel_arr = np.full(nslot, 200.0, np.float32)
        for ww in range(NW):
            for c in range(NCH):
                b = (core * NW + ww) * NCH + c
                lo, hi = bounds[b], bounds[b + 1]
                take = min(Q, hi - lo)
                sel = order[lo : lo + take]
                base = cell_slot[ww, c]
                idx_arr[base : base + take] = (dst[sel] - c * CH).astype(
                    np.int16)
                srel_arr[base : base + take] = srel_v[sel].astype(np.float32)
                if hi - lo > Q:
                    spill_parts.append(order[lo + Q : hi])
        # wrap idx per gather call (call = L*Q consecutive slots)
        blocks = []
        s0 = 0
        for g in cfg.groups:
            L = len(g)
            for c in range(NCH):
                n = L * Q
                blk = idx_arr[s0 : s0 + n]
                blocks.append(np.tile(blk.reshape(-1, 16).T, (8, 1)))
                s0 += n
        idxs = np.ascontiguousarray(np.concatenate(blocks, axis=1))
        srel_T = np.ascontiguousarray(srel_arr.reshape(-1, P).T)
        streams.append({"idxs": idxs, "srel": srel_T})
    spill = (np.concatenate(spill_parts) if spill_parts
             else np.zeros(0, np.int64))
    return streams, src[spill], dst[spill]


# ------------------------------------------------------------ orchestration
def _run(nc, in_maps, cfg, **kw):
    if RUN_MODE == "sim":
        from concourse.bass_interp import MultiCoreSim

        sim = MultiCoreSim(nc, num_cores=cfg.NC, trace=False)
        for ci, core in enumerate(sim.cores.values()):
            for name, arr in in_maps[ci].items():
                core.tensor(name)[:] = arr
        sim.simulate(check_with_hw=False)
        out_names = []
        for alloc in nc.m.functions[0].allocations:
            if not isinstance(alloc, mybir.MemoryLocationSet):
                continue
            if alloc.kind == "ExternalOutput":
                out_names.append(alloc.memorylocations[0].name)
        results = [
            {n: np.array(core.tensor(n)) for n in out_names}
            for core in sim.cores.values()
        ]

        class R:
            pass

        r = R()
        r.results = results
        r.exec_time_ns = None
        return r
    return bass_utils.run_bass_kernel_spmd(
        nc, in_maps, core_ids=list(range(cfg.NC)), **kw
    )


def kernel(x, edge_index, kernel, kernel1, kernel2, gamma, beta, _cfg=None,
           _trace=False):
    cfg = _cfg or CFG
    x = np.asarray(x, np.float32)
    k0 = np.asarray(kernel, np.float32)
    k1 = np.asarray(kernel1, np.float32)
    k2 = np.asarray(kernel2, np.float32)
    gamma = np.asarray(gamma, np.float32)
    beta = np.asarray(beta, np.float32)
    NC, NS, D = cfg.NC, cfg.NS, cfg.D

    import time as _t
    _lap_t = [_t.time()]

    def _lap(msg):
        now = _t.time()
        print(f"[kernel] {msg}: +{now - _lap_t[0]:.1f}s", flush=True)
        _lap_t[0] = now

    # ---- L1
    nc1 = build_l1(cfg)
    _lap("build_l1")
    in1 = [{"x_slice": np.ascontiguousarray(x[c * NS : (c + 1) * NS])}
           for c in range(NC)]
    r1 = _run(nc1, in1, cfg, trace=_trace)
    _lap("run_l1")
    parts = np.stack([r1.results[c]["stats"] for c in range(NC)])
    tot = parts.sum(axis=0).astype(np.float64)
    mean = tot[:, 0] / cfg.N
    var = tot[:, 1] / cfg.N - mean * mean
    rstd = gamma.astype(np.float64) / np.sqrt(var + BN_EPS)
    scale = rstd.astype(np.float32)
    shift = (beta.astype(np.float64) - mean * rstd).astype(np.float32)

    # ---- L2
    nc2 = build_l2(cfg)
    _lap("build_l2")
    in2 = []
    for c in range(NC):
        in2.append({
            "xT_slice": np.ascontiguousarray(x[c * NS : (c + 1) * NS].T),
            "scale": np.ascontiguousarray(scale.reshape(D, 1)),
            "shift": np.ascontiguousarray(shift.reshape(D, 1)),
            "k0": k0, "k1": k1, "k2": k2,
        })
    r2 = _run(nc2, in2, cfg, trace=_trace)
    _lap("run_l2")
    mapped = np.concatenate(
        [np.asarray(r2.results[c]["mappedT"]).T for c in range(NC)], axis=0
    )
    s1 = np.concatenate(
        [np.asarray(r2.results[c]["s1o"]).T.reshape(-1)[:NS] for c in range(NC)]
    )
    s2 = np.concatenate(
        [np.asarray(r2.results[c]["s2o"]).T.reshape(-1)[:NS] for c in range(NC)]
    )

    # ---- host glue: table + edge streams
    tbl = np.zeros((cfg.N, cfg.ROW), ml_dtypes.bfloat16)
    tbl[:, 0:128] = mapped.astype(ml_dtypes.bfloat16)
    tbl[:, 128] = s2.astype(ml_dtypes.bfloat16)
    tbl[:, 129] = 1.0
    streams, sp_src, sp_dst = plan_edges(edge_index, cfg)
    _lap(f"host_glue (spill={len(sp_src)})")

    # ---- L3
    nc3 = build_l3(cfg)
    _lap("build_l3")
    in3 = []
    for c in range(NC):
        s1pad = np.zeros(cfg.NW * P, np.float32)
        s1pad[:NS] = s1[c * NS : (c + 1) * NS]
        in3.append({
            "table": tbl,
            "idxs": streams[c]["idxs"],
            "srel": streams[c]["srel"],
            "s1rows": np.ascontiguousarray(s1pad.reshape(cfg.NW, P)),
        })
    r3 = _run(nc3, in3, cfg, trace=_trace)
    _lap("run_l3")
    raw = np.concatenate(
        [np.asarray(r3.results[c]["out"]) for c in range(NC)], axis=0
    )
    num = raw[:, 0:128].astype(np.float64)
    den = raw[:, 129].astype(np.float64)

    # ---- spill edges on host
    if len(sp_src):
        e = s1[sp_src] + s2[sp_dst]
        el = np.where(e > 0, e, 0.01 * e)
        wsp = np.exp(el).astype(np.float64)
        mb = tbl[:, 0:128].astype(np.float32).astype(np.float64)
        np.add.at(num, sp_src, wsp[:, None] * mb[sp_dst])
        np.add.at(den, sp_src, wsp)

    out = np.maximum(num / np.maximum(den, 1e-16)[:, None], 0.0)
    globals()["_LAST_RESULTS"] = (r1, r2, r3)
    return out.astype(np.float32)
